# revision 24
# baseline (speedup 1.0000x reference)
"""MlpAttentionLayer Trainium2 kernel (v2).

Math (reference):
  cat = [x, x-q, q]                         [B,T,3D]
  h   = BN1(cat); p = relu(h @ W1)          [B,T,D]
  g   = BN2(p);   w = sigmoid(g @ W2)       [B,T,1]
  out = sum_t x * w                         [B,D]

BN1 is affine per-feature, so with s1 = g1/sqrt(v1+eps):
  p_pre = x @ Wx + Qp[b]
    Wx    = s1a*W1a + s1b*W1b           (per-row scaled, [D,D])
    Qp    = q @ Wq + bias0              ([B,D], host precomputed)
BN2+W2 fold to:  logits = relu(p_pre) @ W2p + c2,  W2p = s2*W2, c2 scalar.

Device (per core, 256 batch, groups of G=4):
  x loaded as [100p, b, 2tt, d]  (1KB contiguous per partition line,
  t = 2p+tt), cast fp32->bf16 on GpSimd, 8 PE transposes -> xT in one
  PSUM tile, one fat DVE copy to SBUF.  pre = Wx^T @ xT (N=200/b).
  Qp enters as the per-partition BIAS of the relu (2 relus on ACT via
  activation(bias=QpT col), 2 on DVE via tensor_scalar(add,max)) -- no
  one-hot matmuls.  logits via col-tiled M=32 matmuls, one batched
  sigmoid -> bf16, two bf16 PE transposes of w, one strided extract,
  per-b matvec out_b = x_b^T @ w_b column-packed in PSUM; one fp32
  transpose pair at the end for the [B,D] output layout.
"""

import sys

sys.path.insert(0, "/opt/trn_rl_repo")

import numpy as np
import ml_dtypes

BN_EPS = 1e-3
B, T, D = 2048, 200, 128
N_CORES = 8
BSH = B // N_CORES          # 256 batch elements per core
G = 4                       # batch elements per pipeline group
NGRP = BSH // G             # 64 groups
TH = 100                    # partitions per x tile (t = 2p + tt)
XT_STRIDE = 112             # xT psum free stride per (g,tt) chunk

BF16 = ml_dtypes.bfloat16


def _build_bass():
    from concourse import bacc, mybir
    from concourse.tile import TileContext
    from concourse.masks import make_identity

    fp32 = mybir.dt.float32
    bf16 = mybir.dt.bfloat16
    AF = mybir.ActivationFunctionType
    ALU = mybir.AluOpType

    nc = bacc.Bacc()
    x_d = nc.dram_tensor("x", (BSH, T, D), bf16, kind="ExternalInput")
    qp2_d = nc.dram_tensor("qp2", (2, BSH // 2, D), bf16, kind="ExternalInput")
    sel2_d = nc.dram_tensor("sel2", (2, 2 * T), bf16, kind="ExternalInput")
    sel4_d = nc.dram_tensor("sel4", (D, G), bf16, kind="ExternalInput")
    wx_d = nc.dram_tensor("wx", (D, D), bf16, kind="ExternalInput")
    w2r_d = nc.dram_tensor("w2r", (D, 32), bf16, kind="ExternalInput")
    c2_d = nc.dram_tensor("c2", (1, 1), fp32, kind="ExternalInput")
    out_d = nc.dram_tensor("out", (BSH, D), fp32, kind="ExternalOutput")

    with TileContext(nc) as tc:
        with (
            tc.tile_pool(name="const", bufs=1) as cpool,
            tc.tile_pool(name="x16", bufs=6) as x16pool,
            tc.tile_pool(name="xt", bufs=3) as xtpool,
            tc.tile_pool(name="h1p", bufs=3) as h1pool,
            tc.tile_pool(name="wsbp", bufs=3) as wsbpool,
            tc.tile_pool(name="wtsb", bufs=3) as wtpool_sb,
            tc.tile_pool(name="fin", bufs=1) as finpool,
            tc.tile_pool(name="ps_pre", bufs=1, space="PSUM") as pre_pool,
            tc.tile_pool(name="ps_xt", bufs=1, space="PSUM") as xt_pool,
            tc.tile_pool(name="ps_lw", bufs=1, space="PSUM") as lw_pool,
            tc.tile_pool(name="ps_wt", bufs=1, space="PSUM") as wt_pool,
            tc.tile_pool(name="ps_out", bufs=1, space="PSUM") as fout_pool,
        ):
            ident16 = cpool.tile([128, 128], bf16)
            make_identity(nc, ident16)
            ident32 = cpool.tile([128, 128], fp32)
            make_identity(nc, ident32)
            wx_sb = cpool.tile([D, D], bf16)
            nc.sync.dma_start(wx_sb, wx_d[:, :])
            w2r_sb = cpool.tile([D, 32], bf16)
            nc.sync.dma_start(w2r_sb, w2r_d[:, :])
            c2_sb = cpool.tile([128, 1], fp32)
            nc.sync.dma_start(c2_sb, c2_d[0, 0:1].broadcast_to((128, 1)))
            qp2_sb = cpool.tile([2, BSH // 2, D], bf16)
            nc.sync.dma_start(qp2_sb, qp2_d[:, :, :])
            sel2_sb = cpool.tile([2, 2 * T], bf16)
            nc.sync.dma_start(sel2_sb, sel2_d[:, :])
            sel4_sb = cpool.tile([D, G], bf16)
            nc.sync.dma_start(sel4_sb, sel4_d[:, :])

            fout = fout_pool.tile([128, BSH], mybir.dt.float32)

            # Software-pipelined emission: stage S for group gi is emitted
            # at iteration gi+S, so every PE instruction's inputs are >= 1
            # group old and the (FIFO) PE queue never stalls mid-stream.
            # This keeps the PE HAM-warm (2.4 GHz) -- v4's inline chain ran
            # the whole kernel at the cold 1.2 GHz rate.
            x16s, xts, h1s, wsbs, wts = {}, {}, {}, {}, {}

            def st_load(gi):            # DMA prefetch (one iteration early)
                b0 = gi * G
                x16 = x16pool.tile([TH, G, 2, D], bf16, tag="x16")
                x16s[gi] = x16
                nc.sync.dma_start(
                    x16,
                    x_d[b0 : b0 + G, :, :].rearrange(
                        "b (p tt) d -> p b tt d", tt=2
                    ),
                )

            def st_t_copy(gi):          # xT via regular matmuls + split copies
                xt = xtpool.tile([128, G, 2, TH], bf16, tag="xt")
                xts[gi] = xt
                for pair in range(2):
                    xtp = xt_pool.tile(
                        [128, 2, 2, XT_STRIDE], mybir.dt.float32,
                        tag=f"xtp{pair}",
                    )
                    for sub in range(2):
                        g = 2 * pair + sub
                        for tt in range(2):
                            nc.tensor.matmul(
                                xtp[:, sub, tt, 0:TH],
                                x16s[gi][:, g, tt, :],
                                ident16[0:TH, 0:TH],
                                start=True,
                                stop=True,
                            )
                    src = xtp[:, :, :, 0:TH]
                    dst = xt[:, 2 * pair : 2 * pair + 2, :, :]
                    if pair == 0:
                        nc.vector.tensor_copy(dst, src)
                    else:
                        nc.scalar.activation(dst, src, AF.Copy)

            def st_main_relu(gi):       # pre + one-hot Qp + batched relu
                h1 = h1pool.tile([128, G, T], bf16, tag="h1")
                h1s[gi] = h1
                for pair in range(2):
                    prep = pre_pool.tile(
                        [128, 512], mybir.dt.float32, tag=f"pre{pair}"
                    )
                    nc.tensor.matmul(
                        prep[:, 0 : 2 * T],
                        wx_sb,
                        xts[gi][:, 2 * pair : 2 * pair + 2, :, :].rearrange(
                            "p b t c -> p (b t c)"
                        ),
                        start=True,
                        stop=False,
                    )
                    nc.tensor.matmul(
                        prep[:, 0 : 2 * T],
                        qp2_sb[:, gi * 2 + pair, :],
                        sel2_sb[:, :],
                        start=False,
                        stop=True,
                    )
                    h1_p = h1[:, 2 * pair : 2 * pair + 2, :].rearrange(
                        "p b t -> p (b t)"
                    )
                    if pair == 0:
                        nc.scalar.activation(
                            h1_p, prep[:, 0 : 2 * T], AF.Relu
                        )
                    else:
                        nc.vector.tensor_scalar_max(
                            h1_p, prep[:, 0 : 2 * T], 0.0
                        )

            def st_logits_sig(gi):      # col-tiled logits + batched sigmoid
                lw = lw_pool.tile([128, 512], mybir.dt.float32, tag="lw")
                for g in range(G):
                    nc.tensor.matmul(
                        lw[32 * g : 32 * g + 32, 0:T],
                        w2r_sb,
                        h1s[gi][:, g, :],
                        start=True,
                        stop=True,
                        tile_position=(0, 32 * g),
                    )
                wsb = wsbpool.tile([128, T], bf16, tag="wsb")
                wsbs[gi] = wsb
                nc.scalar.activation(
                    wsb, lw[:, 0:T], AF.Sigmoid, bias=c2_sb[:, 0:1]
                )
                del h1s[gi]

            def st_wt(gi):              # wta/wtb via selector matmuls (N=4)
                wtp = wt_pool.tile([TH, 2, G], mybir.dt.float32, tag="wtp")
                for h in range(2):
                    nc.tensor.matmul(
                        wtp[:, h, :],
                        wsbs[gi][:, h * TH : (h + 1) * TH],
                        sel4_sb,
                        start=True,
                        stop=True,
                    )
                wt = wtpool_sb.tile([TH, 2, G], bf16, tag="wt")
                wts[gi] = wt
                nc.vector.tensor_copy(wt, wtp)
                del wsbs[gi]

            def st_matvec(gi):          # out_b = x_b^T @ w_b, column-packed
                b0 = gi * G
                for g in range(G):
                    bc = b0 + g
                    for tt in range(2):
                        nc.tensor.matmul(
                            fout[:, bc : bc + 1],
                            x16s[gi][:, g, tt, :],
                            wts[gi][:, tt, g : g + 1],
                            start=(tt == 0),
                            stop=(tt == 1),
                        )
                del x16s[gi], xts[gi], wts[gi]

            # (stage, lag) in EMISSION order: main_relu(it-2) is emitted
            # before t_copy(it-1) so the ACT queue runs reluA(it-2) ahead
            # of copyB(it-1) -- otherwise PE's main(it) stalls on the pre
            # WAR behind a queued copy and the HAM never reaches 2.4 GHz.
            sched = [
                (st_load, 0),
                (st_main_relu, 2),
                (st_t_copy, 1),
                (st_logits_sig, 3),
                (st_wt, 4),
                (st_matvec, 5),
            ]
            max_lag = max(lag for _, lag in sched)
            for it in range(NGRP + max_lag):
                for stage, lag in sched:
                    gi = it - lag
                    if 0 <= gi < NGRP:
                        stage(gi)

            # ---- epilogue: transpose [d, b] -> [b, d] and store
            osb = finpool.tile([128, BSH], mybir.dt.float32)
            nc.scalar.activation(osb, fout, AF.Copy)
            obt = finpool.tile([128, BSH], mybir.dt.float32)
            for half in range(2):
                ot = lw_pool.tile([128, 512], mybir.dt.float32, tag="lw")
                nc.tensor.transpose(
                    ot[:, 0:128], osb[:, half * 128 : half * 128 + 128], ident32
                )
                nc.scalar.activation(
                    obt[:, half * 128 : half * 128 + 128], ot[:, 0:128], AF.Copy
                )
                nc.sync.dma_start(
                    out_d[half * 128 : half * 128 + 128, :],
                    obt[:, half * 128 : half * 128 + 128],
                )
    nc.finalize()
    return nc


_NC_CACHE = {}


def _get_nc():
    if "nc" not in _NC_CACHE:
        _NC_CACHE["nc"] = _build_bass()
    return _NC_CACHE["nc"]


def _host_prep(inputs, query, W1, W2,
               bn1_gamma, bn1_beta, bn1_mean, bn1_var,
               bn2_gamma, bn2_beta, bn2_mean, bn2_var):
    x = np.asarray(inputs, np.float32).astype(BF16)
    q = np.asarray(query, np.float64)
    W1 = np.asarray(W1, np.float64)
    W2 = np.asarray(W2, np.float64)
    s1 = np.asarray(bn1_gamma, np.float64) / np.sqrt(
        np.asarray(bn1_var, np.float64) + BN_EPS
    )
    W1s = s1[:, None] * W1                       # scale rows of W1
    Wx = W1s[0:D] + W1s[D : 2 * D]               # [D, D]
    Wq = W1s[2 * D : 3 * D] - W1s[D : 2 * D]     # [D, D]
    bias0 = (np.asarray(bn1_beta, np.float64) - np.asarray(bn1_mean, np.float64) * s1) @ W1
    Qp = q @ Wq + bias0                          # [B, D]
    s2 = np.asarray(bn2_gamma, np.float64) / np.sqrt(
        np.asarray(bn2_var, np.float64) + BN_EPS
    )
    W2p = s2 * W2[:, 0]                          # [D]
    c2 = float(
        (np.asarray(bn2_beta, np.float64) - np.asarray(bn2_mean, np.float64) * s2)
        @ W2[:, 0]
    )

    wx16 = Wx.astype(BF16)                       # lhsT [K=din, M=dout]
    w2r16 = np.repeat(W2p.astype(BF16)[:, None], 32, axis=1)  # [D, 32]
    c2a = np.full((1, 1), c2, np.float32)
    # Qp rows regrouped as [2, n_pairs, D] so a [2, D] slab is the K=2
    # stationary of the one-hot bias matmul.
    qp16 = Qp.astype(BF16).reshape(B // 2, 2, D).transpose(1, 0, 2)
    qp16 = np.ascontiguousarray(qp16)            # [2, B/2, D]
    sel2 = np.zeros((2, 2 * T), BF16)
    sel2[0, 0:T] = 1
    sel2[1, T : 2 * T] = 1
    sel4 = np.zeros((D, G), BF16)
    for g in range(G):
        sel4[32 * g, g] = 1

    in_maps = []
    npair = BSH // 2
    for c in range(N_CORES):
        in_maps.append(
            {
                "x": x[c * BSH : (c + 1) * BSH],
                "qp2": np.ascontiguousarray(
                    qp16[:, c * npair : (c + 1) * npair]
                ),
                "sel2": sel2,
                "sel4": sel4,
                "wx": wx16,
                "w2r": w2r16,
                "c2": c2a,
            }
        )
    return in_maps


def kernel(
    inputs,
    query,
    W1,
    W2,
    bn1_gamma,
    bn1_beta,
    bn1_mean,
    bn1_var,
    bn2_gamma,
    bn2_beta,
    bn2_mean,
    bn2_var,
):
    from concourse.bass_utils import run_bass_kernel_spmd

    in_maps = _host_prep(
        inputs, query, W1, W2,
        bn1_gamma, bn1_beta, bn1_mean, bn1_var,
        bn2_gamma, bn2_beta, bn2_mean, bn2_var,
    )
    nc = _get_nc()
    res = run_bass_kernel_spmd(nc, in_maps, core_ids=list(range(N_CORES)))
    out = np.concatenate([r["out"] for r in res.results], axis=0)
    return out.astype(np.float32)
